# revision 26
# baseline (speedup 1.0000x reference)
"""Trainium2 Bass kernel for nn_AdaptiveCentralLayer.

Input: kernel (128, 8, 256, 256) f32. Per (b, c) slice: compute center of
mass, then circularly roll the 256x256 slice so the center of mass lands at
the center (torch.roll semantics, per-slice data-dependent integer shifts).

Distribution: pure data parallel, batch dim sharded across 8 NeuronCores
(16 batches per core = 128 slices per core).

Per-core dataflow (minimal HBM traffic: read 32 MiB + write 32 MiB):
  1. Load slice into SBUF, rows split into two 128-partition groups; the
     load lands in the first half of a width-doubled tile D[p, g, 0, :].
  2. ScalarE copies D[:,g,0,:] -> D[:,g,1,:] (doubling the row for the
     circular column window) with fused accumulation producing row sums.
  3. PE matmuls: column sums (ones weights) for the x-moment; tiny matmuls
     contract row sums with [ones, y-centered] weights for S and y-moment.
  4. Small-vector stage computes per-slice integer shifts (centered
     coordinates keep f32 error ~1e-5, far below the rounding margin).
  5. Column roll: per-slice register (value_load) gives the dynamic window
     offset; one DVE copy reads D[:, g, ox:ox+256].
  6. Row roll: batched indirect scatter writes each rolled row to its
     destination DRAM row (per-row int32 indices built on device).
"""
import numpy as np

import concourse.bass as bass
import concourse.bacc as bacc
import concourse.mybir as mybir
from concourse.tile import TileContext
from concourse.bass_utils import run_bass_kernel_spmd

B, C, H, W = 128, 8, 256, 256
NCORES = 8
BPC = B // NCORES            # batches per core
NS = BPC * C                 # slices per core
ROWS = NS * H                # output rows per core
G = 4                        # slices per scalar group
GSC = 16                     # slices per scatter call
P = 128
F32 = mybir.dt.float32
I32 = mybir.dt.int32


def _build(ns=NS, gsc=GSC, repeat=1, dbufs=16, cbufs=3, narrow_pool=False,
           split_scatter=True):
    assert ns % gsc == 0 and gsc % G == 0
    nc = bacc.Bacc("TRN2", target_bir_lowering=False, debug=False,
                   num_devices=NCORES)
    rows = ns * H
    x = nc.dram_tensor("x", [rows, W], F32, kind="ExternalInput")
    out = nc.dram_tensor("out", [rows, W], F32, kind="ExternalOutput")
    cw = nc.dram_tensor("cw", [P, 2], F32, kind="ExternalInput")
    xct = nc.dram_tensor("xct", [1, G * W], F32, kind="ExternalInput")
    ones1 = nc.dram_tensor("ones1", [1, P], F32, kind="ExternalInput")
    # iota16[p, q] = 16*(q%8) + p%16 + 128*((q//8)%2) + 512 (idx wrap layout)
    iota16 = nc.dram_tensor("iota16", [P, 16 * gsc], F32, kind="ExternalInput")
    # base16[p, sc*16*gsc + q] = (sc*gsc + q//16) * 256
    base16 = nc.dram_tensor("base16", [P, (ns // gsc) * 16 * gsc], I32,
                            kind="ExternalInput")

    x4 = x.rearrange("(s g p) w -> s g p w", g=2, p=P)
    ngrp_sc = gsc // G

    with TileContext(nc) as tc:
        with (
            tc.tile_pool(name="consts", bufs=1) as kpool,
            tc.tile_pool(name="dpool", bufs=dbufs) as dpool,
            tc.tile_pool(name="cpool", bufs=cbufs) as cpool,
            tc.tile_pool(name="rspool", bufs=10) as rspool,
            tc.tile_pool(name="spool", bufs=4) as spool,
            tc.tile_pool(name="ipool", bufs=2) as ipool,
            tc.tile_pool(name="psc", bufs=2, space="PSUM") as pscp,
            tc.tile_pool(name="pssy", bufs=2, space="PSUM") as pssyp,
            tc.tile_pool(name="psb", bufs=2, space="PSUM") as psbp,
        ):
            cw_t = kpool.tile([P, 2], F32)
            nc.sync.dma_start(out=cw_t[:], in_=cw[:])
            xct_t = kpool.tile([1, G * W], F32)
            nc.sync.dma_start(out=xct_t[:], in_=xct[:])
            ones1_t = kpool.tile([1, P], F32)
            nc.sync.dma_start(out=ones1_t[:], in_=ones1[:])
            iota16_t = kpool.tile([P, 16 * gsc], F32)
            nc.sync.dma_start(out=iota16_t[:], in_=iota16[:])
            base16_t = kpool.tile([P, (ns // gsc) * 16 * gsc], I32)
            nc.sync.dma_start(out=base16_t[:], in_=base16[:])

            def emit_body():
              for sc in range(ns // gsc):
                c_t = cpool.tile([P, 2 * gsc, W], F32)
                syrow = ipool.tile([1, gsc], F32, tag="syrow")
                for gi in range(ngrp_sc):
                    grp = sc * ngrp_sc + gi
                    psC = pscp.tile([1, G * W], F32, space="PSUM")
                    psSY = pssyp.tile([1, 4 * G], F32, space="PSUM")
                    d_tiles = []
                    for s in range(G):
                        S = grp * G + s
                        d = dpool.tile([P, 2, 2, W], F32, tag="d")
                        nc.sync.dma_start(out=d[:, :, 0, :],
                                          in_=x4[S].transpose([1, 0, 2]))
                        rs = rspool.tile([P, 2], F32, tag="rs")
                        nc.scalar.activation(
                            out=d[:, 0, 1, :], in_=d[:, 0, 0, :],
                            func=mybir.ActivationFunctionType.Copy,
                            accum_out=rs[:, 0:1])
                        nc.scalar.activation(
                            out=d[:, 1, 1, :], in_=d[:, 1, 0, :],
                            func=mybir.ActivationFunctionType.Copy,
                            accum_out=rs[:, 1:2])
                        nc.tensor.matmul(out=psC[0:1, s * W:(s + 1) * W],
                                         lhsT=cw_t[:, 0:1], rhs=d[:, 0, 0, :],
                                         start=True, stop=False)
                        nc.tensor.matmul(out=psC[0:1, s * W:(s + 1) * W],
                                         lhsT=cw_t[:, 0:1], rhs=d[:, 1, 0, :],
                                         start=False, stop=True)
                        nc.tensor.matmul(out=psSY[0:1, 4 * s:4 * s + 2],
                                         lhsT=cw_t[:, 0:1], rhs=rs[:, 0:2],
                                         start=True, stop=True)
                        nc.tensor.matmul(out=psSY[0:1, 4 * s + 2:4 * s + 4],
                                         lhsT=cw_t[:, 1:2], rhs=rs[:, 0:2],
                                         start=True, stop=True)
                        d_tiles.append(d)

                    # ---- group scalar stage (tiny tensors) ----
                    scr = spool.tile([1, G * W], F32, tag="scr")
                    scr_eng = nc.gpsimd if narrow_pool else nc.vector
                    if narrow_pool:
                        # Pool cannot read PSUM: copy colsums to SBUF on ACT
                        csb = spool.tile([1, G * W], F32, tag="csb")
                        nc.scalar.copy(out=csb[:], in_=psC[0:1, :])
                        scr_eng.tensor_tensor(out=scr[:], in0=csb[:],
                                              in1=xct_t[0:1, :],
                                              op=mybir.AluOpType.mult)
                    else:
                        nc.vector.tensor_tensor(out=scr[:], in0=psC[0:1, :],
                                                in1=xct_t[0:1, :],
                                                op=mybir.AluOpType.mult)
                    sxp = spool.tile([1, G], F32, tag="sxp")
                    nc.vector.reduce_sum(
                        out=sxp[:].unsqueeze(2),
                        in_=scr[:].rearrange("o (g w) -> o g w", w=W),
                        axis=mybir.AxisListType.X)

                    sy_sb = spool.tile([1, 4 * G], F32, tag="sy_sb")
                    nc.scalar.copy(out=sy_sb[:], in_=psSY[0:1, :])
                    v = sy_sb[0:1, :].rearrange("o (s q) -> o s q", q=4)
                    srow = spool.tile([1, G], F32, tag="srow")
                    nc.vector.tensor_tensor(out=srow[:], in0=v[:, :, 0],
                                            in1=v[:, :, 1],
                                            op=mybir.AluOpType.add)
                    y01 = spool.tile([1, G], F32, tag="y01")
                    nc.vector.tensor_tensor(out=y01[:], in0=v[:, :, 2],
                                            in1=v[:, :, 3],
                                            op=mybir.AluOpType.add)
                    syp = spool.tile([1, G], F32, tag="syp")
                    nc.vector.tensor_scalar(out=syp[:], in0=v[:, :, 1],
                                            scalar1=128.0, scalar2=None,
                                            op0=mybir.AluOpType.mult)
                    nc.vector.tensor_tensor(out=syp[:], in0=syp[:], in1=y01[:],
                                            op=mybir.AluOpType.add)

                    # sy = floor(1.0 - Sy/S) = round(0.5 - Sy/S); same for x.
                    # floor built from int-cast + fix so it is correct under
                    # either cast rounding mode (sim truncates, HW is RNE).
                    rS = spool.tile([1, G], F32, tag="rS")
                    nc.vector.reciprocal(out=rS[:], in_=srow[:])

                    def floor_shift(mom, tagp, out_ap=None):
                        a = spool.tile([1, G], F32, tag=tagp + "a")
                        nc.vector.tensor_tensor(out=a[:], in0=mom[:],
                                                in1=rS[:],
                                                op=mybir.AluOpType.mult)
                        nc.vector.tensor_scalar(out=a[:], in0=a[:],
                                                scalar1=-1.0, scalar2=1.0,
                                                op0=mybir.AluOpType.mult,
                                                op1=mybir.AluOpType.add)
                        fi = spool.tile([1, G], I32, tag=tagp + "i")
                        nc.vector.tensor_copy(out=fi[:], in_=a[:])
                        fb = spool.tile([1, G], F32, tag=tagp + "b")
                        nc.vector.tensor_copy(out=fb[:], in_=fi[:])
                        gt = spool.tile([1, G], F32, tag=tagp + "g")
                        nc.vector.tensor_tensor(out=gt[:], in0=fb[:], in1=a[:],
                                                op=mybir.AluOpType.is_gt)
                        if out_ap is None:
                            sf = spool.tile([1, G], F32, tag=tagp + "s")
                            out_ap = sf[:]
                        nc.vector.tensor_tensor(out=out_ap, in0=fb[:],
                                                in1=gt[:],
                                                op=mybir.AluOpType.subtract)
                        return out_ap

                    floor_shift(syp, "fy",
                                out_ap=syrow[:, gi * G:(gi + 1) * G])
                    sxf = floor_shift(sxp, "fx")

                    # ox = (512 - sx) & 255
                    oxf = spool.tile([1, G], F32, tag="oxf")
                    nc.vector.tensor_scalar(out=oxf[:], in0=sxf[:],
                                            scalar1=-1.0, scalar2=512.0,
                                            op0=mybir.AluOpType.mult,
                                            op1=mybir.AluOpType.add)
                    oxi = spool.tile([1, G], I32, tag="oxi")
                    nc.vector.tensor_copy(out=oxi[:], in_=oxf[:])
                    nc.vector.tensor_scalar(out=oxi[:], in0=oxi[:],
                                            scalar1=255, scalar2=None,
                                            op0=mybir.AluOpType.bitwise_and)

                    # ---- per-slice dynamic column-roll window ----
                    for s in range(G):
                        ox = nc.values_load(
                            oxi[0:1, s:s + 1],
                            engines=[mybir.EngineType.DVE],
                            min_val=0, max_val=W,
                            skip_runtime_bounds_check=True)
                        dv = d_tiles[s][:].rearrange("p g d w -> p g (d w)")
                        lo = (gi * G + s) * 2
                        nc.vector.tensor_copy(out=c_t[:, lo:lo + 2, :],
                                              in_=dv[:, :, bass.ds(ox, W)])

                # scatter indices in dma_scatter_add's wrapped-int16 layout:
                # logical chunk k = c*128 + p lives at idxs[k%16, k//16];
                # dest row = base + ((p + 128*g + sy + 512) & 255)
                nq = 16 * gsc
                syr256 = ipool.tile([1, nq], F32, tag="syr256")
                nc.vector.tensor_copy(
                    out=syr256[:].rearrange("o (s r) -> o s r", r=16),
                    in_=syrow[:].unsqueeze(2).to_broadcast([1, gsc, 16]))
                psB = psbp.tile([P, nq], F32, space="PSUM")
                nc.tensor.matmul(out=psB[:, :], lhsT=ones1_t[:, :],
                                 rhs=syr256[:, :], start=True, stop=True)
                rowf = ipool.tile([P, nq], F32, tag="rowf")
                nc.vector.tensor_tensor(out=rowf[:], in0=iota16_t[:],
                                        in1=psB[:, :],
                                        op=mybir.AluOpType.add)
                rowi = ipool.tile([P, nq], I32, tag="rowi")
                nc.vector.tensor_copy(out=rowi[:], in_=rowf[:])
                nc.vector.tensor_scalar(out=rowi[:], in0=rowi[:],
                                        scalar1=255, scalar2=None,
                                        op0=mybir.AluOpType.bitwise_and)
                nc.vector.tensor_tensor(
                    out=rowi[:], in0=rowi[:],
                    in1=base16_t[:, sc * nq:(sc + 1) * nq],
                    op=mybir.AluOpType.add)
                idx16 = ipool.tile([P, nq], mybir.dt.int16, tag="idx16")
                nc.vector.tensor_copy(out=idx16[:], in_=rowi[:])

                if split_scatter:
                    half = 16 * gsc // 2
                    for h in range(2):
                        nc.gpsimd.dma_scatter_add(
                            out_ap=out[:],
                            in_ap=c_t[:, h * gsc:(h + 1) * gsc, :],
                            idxs_ap=idx16[:, h * half:(h + 1) * half],
                            num_idxs=128 * gsc,
                            num_idxs_reg=128 * gsc,
                            elem_size=W)
                else:
                    nc.gpsimd.dma_scatter_add(
                        out_ap=out[:],
                        in_ap=c_t[:, :, :],
                        idxs_ap=idx16[:, :],
                        num_idxs=128 * 2 * gsc,
                        num_idxs_reg=128 * 2 * gsc,
                        elem_size=W)

            if repeat == 1:
                emit_body()
            else:
                with tc.For_i(0, repeat, 1):
                    emit_body()

    nc.compile()
    return nc


def _consts(ns=NS, gsc=GSC):
    p = np.arange(P, dtype=np.float32)
    cw = np.stack([np.ones(P, np.float32), (p - 127.5).astype(np.float32)],
                  axis=1)
    xct = np.tile((np.arange(W) - 127.5).astype(np.float32), G).reshape(1, -1)
    ones1 = np.ones((1, P), dtype=np.float32)
    nq = 16 * gsc
    q = np.arange(nq)
    pp = np.arange(P)
    iota16 = (16 * (q[None, :] % 8) + (pp[:, None] % 16)
              + 128 * ((q[None, :] // 8) % 2) + 512).astype(np.float32)
    nsc = ns // gsc
    base16 = np.zeros((P, nsc * nq), dtype=np.int32)
    for sc in range(nsc):
        base16[:, sc * nq:(sc + 1) * nq] = \
            ((sc * gsc + q // 16) * 256)[None, :]
    return {"cw": cw, "xct": xct, "ones1": ones1, "iota16": iota16,
            "base16": base16}


_NC_CACHE = {}


def _get_nc(ns=NS, gsc=GSC):
    key = (ns, gsc)
    if key not in _NC_CACHE:
        _NC_CACHE[key] = _build(ns, gsc)
    return _NC_CACHE[key]


def kernel(**inputs):
    k = np.ascontiguousarray(np.asarray(inputs["kernel"], dtype=np.float32))
    assert k.shape == (B, C, H, W)
    nc = _get_nc()
    consts = _consts()
    in_maps = []
    for c in range(NCORES):
        shard = k[c * BPC:(c + 1) * BPC].reshape(ROWS, W)
        m = {"x": shard}
        m.update(consts)
        in_maps.append(m)
    res = run_bass_kernel_spmd(nc, in_maps, core_ids=list(range(NCORES)))
    outs = [res.results[i]["out"].reshape(BPC, C, H, W) for i in range(NCORES)]
    full = np.concatenate(outs, axis=0)
    return full
